# revision 61
# baseline (speedup 1.0000x reference)
"""Trainium2 Bass kernel for nn_CachedMLP (2-expert dense MoE MLP).

Computation (reference):
    ew = expert_weights, swapped if expert_ids[0] != 0
    for e in {0,1}:  down_e = (silu(x @ w1_e.T) * (x @ w3_e.T)) @ w2_e
    out = down_0 * ew[0] + down_1 * ew[1]

Sharding: expert-parallel x tensor-parallel. Core c handles expert c//4
and rows [r*2867, (r+1)*2867) of that expert's w1/w3/w2 (r = c%4),
zero-padded to 2944 = 23*128. ew[e] (and the fp8 weight scale) is
folded into w2 on the host, so the sum of the 8 per-core partial
outputs is the final result.

Precision: w1/w3 are stored as fp8-E3M4 (scaled x64 on host; the PE
upconverts exactly, and silu de-scales via activation's scale=1/64);
w2, x, h stay bf16. Measured end-to-end rel err ~1.6% (tolerance 2e-2).
This halves the w13 HBM traffic: per-core reads drop 71.5 -> ~48 MB,
and the kernel is HBM-wire bound.

Device kernel per core:
  pass 1 over 6 a-slices (512 wide, last 384): for each d-chunk kd,
      gate[t, a-slice] += xT(kd) .T-matmul w1_tile(kd)   (PSUM, N=512)
      up[t, a-slice]   += xT(kd) .T-matmul w3_tile(kd)
    (x is the stationary operand -- loaded once per kd, amortized over
    both matmuls; the moving operand is the fp8 weight tile at N=512,
    so the PE streams near peak instead of reloading weights per MM).
    Then silu(scale=1/64)*up -> h[t, a-slice] (bf16), and PE-transpose
    128-col chunks into hT[a, t] for pass 2.
  pass 2: down[t, d-quarter] += hT(ka).T-matmul w2_tile(ka, q),
    accumulated over all 23 ka in 2 PSUM banks per 1024-wide quarter,
    then copied to SBUF and DMA'd out as bf16 partials.

DMA ring: everything data-sized (x, w13, w2, out) rides the single SP
(sync) HWDGE ring, so program-emission order IS the wire priority
order -- per slice: w13 groups first, then that slice's w2 quads.
Round-robin between two rings would steal w13 bandwidth during pass 1
(all per-core queues share one DMA engine anyway).
"""

import json
import os

import ml_dtypes
import numpy as np

T = 128          # tokens
D = 4096         # hidden dim
ACTIVE = 11468   # sparsity-selected neurons per expert
NCORES = 8
ASH = ACTIVE // 4        # 2867 active rows per core
NKA = 23                 # 128-row a-chunks per core
APAD = NKA * 128         # 2944
NKD = D // 128           # 32 d-chunks
JW_LAST = ASH - (NKA - 1) * 128  # 51 useful rows in the last a-chunk
NSL = 6                  # a-slices in pass 1
SLW = 512                # slice width (last slice: 384)
SLW_LAST = APAD - 5 * SLW  # 384
NQ = 4                   # pass-2 d-quarters
QW = D // NQ             # 1024

WSCALE = 64.0            # fp8 weight scale for w1/w3
K8 = 14                  # leading 128-row a-chunks of w2 stored e3m4
KB16 = NKA - K8          # trailing chunks kept bf16 (8 paired + last jw=51)

BF16 = ml_dtypes.bfloat16
E3M4 = ml_dtypes.float8_e3m4

_EVENTSEM_CAP = 2


def _split_multi_waits(bir_json: bytes) -> bytes:
    """Hoist excess per-instruction sync waits into standalone waits.

    The axon-path walrus build accepts at most 1 sync-wait command per
    instruction (2 for EventSemaphore); Tile's wait assigner can emit
    more. Extra waits become wait-only EventSemaphore instructions
    inserted just before the offender on the same engine stream, which
    preserves semantics (the engine would have blocked there anyway).
    """
    d = json.loads(bir_json)
    for func in d.get("functions", []):
        for blk in func.get("blocks", []):
            out = []
            for inst in blk.get("instructions", []):
                sync = inst.get("sync_info")
                waits = (sync or {}).get("on_wait") or []
                cap = _EVENTSEM_CAP if inst.get("opcode") == "EventSemaphore" else 1
                if len(waits) > cap:
                    extra, keep = waits[:-cap], waits[-cap:]
                    for j in range(0, len(extra), _EVENTSEM_CAP):
                        w_inst = {
                            "engine": inst["engine"],
                            "ins": [],
                            "name": f"{inst['name']}-hw{j}",
                            "opcode": "EventSemaphore",
                            "outs": [],
                            "sync_info": {
                                "on_update": [],
                                "on_wait": extra[j : j + _EVENTSEM_CAP],
                            },
                        }
                        if "debug" in inst:
                            w_inst["debug"] = inst["debug"]
                        out.append(w_inst)
                    sync["on_wait"] = keep
                out.append(inst)
            blk["instructions"] = out
    return json.dumps(d).encode()


def _hoist_head_dmas(bir_json: bytes, max_hoist: int = 4) -> bytes:
    """Move the leading wait-free SP DMACopies to the head of the preamble.

    Tile's prologue (engine reg-init + const memsets + all-engine
    barrier rounds) takes ~8.5us before the first dma_start issues,
    leaving the HBM wire idle. The first input DMAs have no waits and
    their destination tiles are disjoint from everything the preamble
    writes (asserted below), so issuing them at the very front of the
    SP stream is safe: DGE rings and DMA semaphores are initialized by
    the runtime at NEFF load (not by the preamble), HWDGE keeps
    per-engine FIFO order, and the sem increments are only ever waited
    on with >= thresholds.
    """
    d = json.loads(bir_json)
    for func in d.get("functions", []):
        blocks = func.get("blocks", [])
        if len(blocks) < 2:
            continue
        main, tile_blk = blocks[0], blocks[1]
        if main.get("name") != "main" or not tile_blk.get("name", "").startswith(
            "tile_context"
        ):
            continue
        # preamble must write only const tiles, else hoisting is unsafe
        pre_outs = {
            o.get("memref")
            for inst in main["instructions"]
            for o in inst.get("outs", [])
            if isinstance(o, dict)
        }
        if any(m and not m.startswith("const-") for m in pre_outs):
            continue
        hoisted = []
        remaining = []
        for inst in tile_blk["instructions"]:
            if (
                len(hoisted) < max_hoist
                and inst.get("engine") == "SP"
                and inst.get("opcode") == "DMACopy"
                and not ((inst.get("sync_info") or {}).get("on_wait"))
            ):
                hoisted.append(inst)
            else:
                remaining.append(inst)
        if not hoisted:
            continue
        main["instructions"][0:0] = hoisted
        tile_blk["instructions"] = remaining
    return json.dumps(d).encode()


def _install_wait_split():
    import concourse.bass2jax as b2j
    import concourse.bass_utils as bu

    if getattr(bu.compile_bir_kernel, "_wait_split", False):
        return
    orig = bu.compile_bir_kernel

    def compile_with_split(bir_json, tmpdir, neff_name="file.neff"):
        return orig(_split_multi_waits(_hoist_head_dmas(bir_json)), tmpdir, neff_name)

    compile_with_split._wait_split = True
    bu.compile_bir_kernel = compile_with_split
    if getattr(b2j, "compile_bir_kernel", None) is orig:
        b2j.compile_bir_kernel = compile_with_split


_program = None


def _build_program():
    """Build the single-core Bass/Tile program (same program on all 8 cores)."""
    import concourse.bass as bass
    import concourse.mybir as mybir
    from concourse.tile import TileContext

    f32 = mybir.dt.float32
    bf16 = mybir.dt.bfloat16
    fp8e3 = mybir.dt.float8e3
    Silu = mybir.ActivationFunctionType.Silu

    nc = bass.Bass()
    xb = nc.declare_dram_parameter("xb", [128, D], bf16, isOutput=False)
    w13a = nc.declare_dram_parameter(
        "w13a", [NSL - 1, NKD // 8, 128, 16 * SLW], fp8e3, isOutput=False
    )
    w13b = nc.declare_dram_parameter(
        "w13b", [NKD // 8, 128, 16 * SLW_LAST], fp8e3, isOutput=False
    )
    w2q8q = nc.declare_dram_parameter(
        "w2q8q", [NQ, 3, 128, 4 * QW], fp8e3, isOutput=False
    )
    w2q8p = nc.declare_dram_parameter(
        "w2q8p", [NQ, 128, 2 * QW], fp8e3, isOutput=False
    )
    w2qbq = nc.declare_dram_parameter(
        "w2qbq", [NQ, 2, 128, 4 * QW], bf16, isOutput=False
    )
    w2ql = nc.declare_dram_parameter("w2ql", [NQ, 128, QW], bf16, isOutput=False)
    oscale = nc.declare_dram_parameter("oscale", [128, 1], f32, isOutput=False)
    out = nc.declare_dram_parameter("out", [T, D], bf16, isOutput=True)

    def slw_of(s):
        return SLW_LAST if s == NSL - 1 else SLW

    def jw_of(ka):
        return JW_LAST if ka == NKA - 1 else 128

    with TileContext(nc) as tc:
        with (
            tc.tile_pool(name="singles", bufs=1) as singles,
            tc.tile_pool(name="w13p", bufs=4) as w13p,
            tc.tile_pool(name="w2p", bufs=14) as w2p,
            tc.tile_pool(name="gap", bufs=2) as gap,
            tc.tile_pool(name="hp", bufs=2) as hp,
            tc.tile_pool(name="outp", bufs=2) as outp,
            tc.tile_pool(name="psum_ug", bufs=2, space="PSUM") as psum_ug,
            tc.tile_pool(name="psum_o", bufs=2, space="PSUM") as psum_o,
        ):
            # x on the SP ring, split into quarters: d-chunk group g only
            # needs quarter g, so quarter 0 goes first and quarters 1-3
            # are interleaved ahead of their w13 group in slice 0 below.
            # This gets the first matmul going after ~1.3 MB of wire.
            xb_s = singles.tile([128, D], bf16)

            def xb_quarter(xq):
                nc.sync.dma_start(
                    out=xb_s[:, xq * (D // 4) : (xq + 1) * (D // 4)],
                    in_=xb[:, xq * (D // 4) : (xq + 1) * (D // 4)],
                )

            xb_quarter(0)
            osc_s = singles.tile([128, 1], f32)
            nc.scalar.dma_start(out=osc_s, in_=oscale[:, :])
            hT = singles.tile([128, APAD], bf16)

            # pass-2 plumbing, shared between the interleaved quarter 0
            # (emitted inside pass 1 as its hT chunks appear) and the
            # trailing quarters 1-3. w2 arrives in 4-ka quads aligned to
            # pass-1 slices; the two bf16 quads straddle slices, so tiles
            # are held across slice boundaries in `held`.
            w2cache = {}
            held = {}

            def w2dma(q, kind, idx):
                if kind == "e4":
                    t = w2p.tile([128, 4 * QW], fp8e3)
                    nc.sync.dma_start(out=t, in_=w2q8q[q, idx, :, :])
                elif kind == "ep":
                    t = w2p.tile([128, 2 * QW], fp8e3)
                    nc.sync.dma_start(out=t, in_=w2q8p[q, :, :])
                elif kind == "b4":
                    t = w2p.tile([128, 4 * QW], bf16)
                    nc.sync.dma_start(out=t, in_=w2qbq[q, idx, :, :])
                else:  # "last" (ka 22, jw rows only)
                    t = w2p.tile([128, QW], bf16)
                    nc.sync.dma_start(out=t[:JW_LAST], in_=w2ql[q, :JW_LAST, :])
                return t

            SLICE_FETCH = {
                0: [("e4", 0)],
                1: [("e4", 1)],
                2: [("e4", 2)],
                3: [("ep", 0), ("b4", 0)],
                4: [("b4", 1)],
                5: [("last", 0)],
            }

            def get_tile(q, kind, idx):
                return w2cache.pop((q, kind, idx), None) or w2dma(q, kind, idx)

            def prefetch_slice(q, s):
                for kind, idx in SLICE_FETCH[s]:
                    w2cache[(q, kind, idx)] = w2dma(q, kind, idx)

            def emit_slice_mms(q, ops, s):
                def mm2(w2t, ka, colbase):
                    lhsT = hT[:, ka * 128 : (ka + 1) * 128]
                    for b in range(2):
                        nc.tensor.matmul(
                            ops[:, b * 512 : (b + 1) * 512],
                            lhsT,
                            w2t[:, colbase + b * 512 : colbase + (b + 1) * 512],
                            start=(ka == 0),
                            stop=False,
                            skip_group_check=True,
                        )

                if s <= 2:
                    t = get_tile(q, "e4", s)
                    for m in range(4):
                        mm2(t, 4 * s + m, m * QW)
                elif s == 3:
                    t = get_tile(q, "ep", 0)
                    mm2(t, 12, 0)
                    mm2(t, 13, QW)
                    tb = get_tile(q, "b4", 0)
                    held[(q, 0)] = tb
                    mm2(tb, 14, 0)
                    mm2(tb, 15, QW)
                elif s == 4:
                    tb = held.pop((q, 0))
                    mm2(tb, 16, 2 * QW)
                    mm2(tb, 17, 3 * QW)
                    tb1 = get_tile(q, "b4", 1)
                    held[(q, 1)] = tb1
                    mm2(tb1, 18, 0)
                    mm2(tb1, 19, QW)
                else:
                    tb1 = held.pop((q, 1))
                    mm2(tb1, 20, 2 * QW)
                    mm2(tb1, 21, 3 * QW)
                    tl = get_tile(q, "last", 0)
                    lhsT = hT[:JW_LAST, (NKA - 1) * 128 : NKA * 128]
                    for b in range(2):
                        nc.tensor.matmul(
                            ops[:, b * 512 : (b + 1) * 512],
                            lhsT,
                            tl[:JW_LAST, b * 512 : (b + 1) * 512],
                            start=False,
                            stop=True,
                            skip_group_check=True,
                        )

            def emit_store(q, ops):
                oth = outp.tile([T, QW], bf16, name=f"oth{q}", tag="oth")
                nc.vector.tensor_scalar_mul(out=oth, in0=ops, scalar1=osc_s)
                # output store on the SP ring (idle once w13 is done):
                # doesn't queue behind the remaining w2 transfers on ACT
                nc.sync.dma_start(out=out[:, q * QW : (q + 1) * QW], in_=oth)

            # pass 1: gate/up in [t, a] layout, x stationary, fp8 w13 moving.
            # w13 arrives in 8-kd batches (1 MB, 8 KB partition lines).
            # Quarter 0 of the down projection rides along: its ka-pair
            # matmuls are emitted right after the slice that produced
            # their hT chunks, filling pass 1's DMA-wait gaps on the PE.
            ops_q0 = psum_o.tile([128, QW], f32, name="o0", tag="od")
            ops_q1 = psum_o.tile([128, QW], f32, name="o1", tag="od")
            for s in range(NSL):
                sw = slw_of(s)
                gate_ps = psum_ug.tile([128, sw], f32, name=f"g{s}", tag="gate")
                up_ps = psum_ug.tile([128, sw], f32, name=f"u{s}", tag="up")
                for g in range(NKD // 8):
                    if s == 0 and g >= 1:
                        xb_quarter(g)
                    w13t = w13p.tile([128, 16 * sw], fp8e3)
                    if s == 0 and g == 0:
                        # split the very first tile so the first matmuls
                        # start after 512 KB instead of 1 MB
                        hw_ = 8 * sw
                        nc.sync.dma_start(
                            out=w13t[:, :hw_], in_=w13a[0, 0, :, :hw_]
                        )
                        nc.sync.dma_start(
                            out=w13t[:, hw_:], in_=w13a[0, 0, :, hw_:]
                        )
                    elif s < NSL - 1:
                        nc.sync.dma_start(out=w13t, in_=w13a[s, g, :, :])
                    else:
                        nc.sync.dma_start(out=w13t, in_=w13b[g, :, :])
                    for m in range(8):
                        kd = 8 * g + m
                        lhsT = xb_s[:, kd * 128 : (kd + 1) * 128]
                        nc.tensor.matmul(
                            gate_ps,
                            lhsT,
                            w13t[:, m * 2 * sw : m * 2 * sw + sw],
                            start=(kd == 0),
                            stop=(kd == NKD - 1),
                        )
                        nc.tensor.matmul(
                            up_ps,
                            lhsT,
                            w13t[:, m * 2 * sw + sw : (m + 1) * 2 * sw],
                            start=(kd == 0),
                            stop=(kd == NKD - 1),
                        )
                ga = gap.tile([128, sw], f32)
                nc.scalar.activation(
                    out=ga, in_=gate_ps, func=Silu, scale=1.0 / WSCALE
                )
                hsl = hp.tile([128, sw], bf16)
                nc.vector.tensor_mul(out=hsl, in0=ga, in1=up_ps)
                # hT chunks via DVE 32x32 block transposes, written straight
                # into hT. Not the PE (keeps it free for matmuls) and not
                # the DMA XBAR (a 128x128 XBAR transpose costs ~1.25us of
                # DMA-engine time, and the single shared DMA engine is the
                # kernel's scarcest resource).
                for c in range(sw // 128):
                    ka = s * 4 + c
                    for bi in range(4):
                        for bj in range(4):
                            nc.vector.transpose(
                                out=hT[
                                    32 * bj : 32 * bj + 32,
                                    ka * 128 + 32 * bi : ka * 128 + 32 * bi + 32,
                                ],
                                in_=hsl[
                                    32 * bi : 32 * bi + 32,
                                    c * 128 + 32 * bj : c * 128 + 32 * bj + 32,
                                ],
                            )
                # quarters 0 and 1 of the down projection ride inside
                # pass 1 (PE work to fill DMA-wait gaps, and their w2
                # consumption keeps the wire schedule balanced)
                emit_slice_mms(0, ops_q0, s)
                emit_slice_mms(1, ops_q1, s)
            emit_store(0, ops_q0)
            emit_store(1, ops_q1)

            # pass 2 remainder: quarters 2-3, each accumulating across all
            # ka in 2 PSUM banks (rotating through the bufs=2 pool), output
            # de-scale copy + DMA overlapping the next quarter's compute.
            for q in range(2, NQ):
                ops = psum_o.tile([128, QW], f32, name=f"o{q}", tag="od")
                for s in range(NSL):
                    emit_slice_mms(q, ops, s)
                emit_store(q, ops)

    return nc


def _pack_w13(w1s: np.ndarray, w3s: np.ndarray):
    """[ASH, D] f32 pair -> ([NSL-1, 4, 128, 16*SLW], [4, 128, 16*SLW_LAST])
    e3m4 blobs, scaled by WSCALE, batched 8 d-chunks per 1 MiB transfer
    (8 KB partition lines, the shape that sustains full DMA throughput).

    blob[s, g, p, m*2*sw + w*sw + j] = w{1,3}s[s*SLW + j, (8g+m)*128 + p].
    """
    padded = np.zeros((2, NSL * SLW, D), dtype=E3M4)
    padded[0, :ASH] = (w1s * np.float32(WSCALE)).astype(E3M4)
    padded[1, :ASH] = (w3s * np.float32(WSCALE)).astype(E3M4)
    # [2, NSL, SLW, NKD, 128] -> [s, kd, p, w, j]
    r = padded.reshape(2, NSL, SLW, NKD, 128).transpose(1, 3, 4, 0, 2)
    # [s, g, m, p, w, j] -> [s, g, p, m, w, j]
    r = r.reshape(NSL, NKD // 8, 8, 128, 2, SLW).transpose(0, 1, 3, 2, 4, 5)
    a = np.ascontiguousarray(r[: NSL - 1]).reshape(NSL - 1, NKD // 8, 128, 16 * SLW)
    b = np.ascontiguousarray(r[NSL - 1, :, :, :, :, :SLW_LAST]).reshape(
        NKD // 8, 128, 16 * SLW_LAST
    )
    return a, b


def _pack_w2(w2s: np.ndarray, scale: float):
    """[ASH, D] f32 -> (e3m4 4-ka quads, e3m4 2-ka pair, bf16 4-ka quads,
    last bf16 chunk) blobs, layout [q, idx, p, m*QW + j].

    scale (= ew * S2 / WSCALE) is folded in; ka < 12 e3m4 quads, ka 12-13
    an e3m4 pair, ka 14-21 bf16 quads, the last (jw=51) chunk on its own
    so its DMA can be row-trimmed.
    """
    v = w2s * np.float32(scale)
    eq = v[: 12 * 128].astype(E3M4)
    a = np.ascontiguousarray(
        eq.reshape(3, 4, 128, NQ, QW).transpose(3, 0, 2, 1, 4)
    ).reshape(NQ, 3, 128, 4 * QW)
    ep = v[12 * 128 : 14 * 128].astype(E3M4)
    p = np.ascontiguousarray(
        ep.reshape(2, 128, NQ, QW).transpose(2, 1, 0, 3)
    ).reshape(NQ, 128, 2 * QW)
    bq = v[14 * 128 : 22 * 128].astype(BF16)
    b = np.ascontiguousarray(
        bq.reshape(2, 4, 128, NQ, QW).transpose(3, 0, 2, 1, 4)
    ).reshape(NQ, 2, 128, 4 * QW)
    last = np.zeros((128, D), dtype=BF16)
    last[:JW_LAST] = v[(NKA - 1) * 128 :].astype(BF16)
    l = np.ascontiguousarray(last.reshape(128, NQ, QW).transpose(1, 0, 2))
    return a, p, b, l


def _pack_x(x: np.ndarray) -> np.ndarray:
    """[T, D] f32 -> [128, D] bf16: xb[p, kd*128 + t] = x[t, kd*128 + p]."""
    return (
        x.astype(BF16).reshape(T, NKD, 128).transpose(2, 1, 0).reshape(128, NKD * T)
    )


def make_in_maps(
    hidden_states,
    expert_weights,
    expert_ids,
    w1_e0,
    w3_e0,
    w2_e0,
    w1_e1,
    w3_e1,
    w2_e1,
):
    ids = np.asarray(expert_ids).reshape(-1)
    ew = np.asarray(expert_weights, dtype=np.float32).reshape(-1)
    if int(ids[0]) != 0:
        ew = ew[::-1]

    xb = _pack_x(np.asarray(hidden_states, dtype=np.float32))
    w1 = (np.asarray(w1_e0, np.float32), np.asarray(w1_e1, np.float32))
    w3 = (np.asarray(w3_e0, np.float32), np.asarray(w3_e1, np.float32))
    w2 = (np.asarray(w2_e0, np.float32), np.asarray(w2_e1, np.float32))

    in_maps = []
    for core in range(NCORES):
        e, r = divmod(core, 4)
        rows = slice(r * ASH, (r + 1) * ASH)
        w13a, w13b = _pack_w13(w1[e][rows], w3[e][rows])
        # h is scaled by WSCALE (up de-scale folded here), so w2 gets
        # ew / WSCALE, plus S2 to center the e3m4 chunks in range
        # (sigma_eff ~ 2); S2 is undone by oscale on the output copy.
        ewe = float(ew[e])
        s2 = 2.0 ** np.round(np.log2(6400.0 / max(abs(ewe), 1e-6)))
        w2q8q, w2q8p, w2qbq, w2ql = _pack_w2(w2[e][rows], ewe * s2 / WSCALE)
        in_maps.append(
            {
                "xb": xb,
                "w13a": w13a,
                "w13b": w13b,
                "w2q8q": w2q8q,
                "w2q8p": w2q8p,
                "w2qbq": w2qbq,
                "w2ql": w2ql,
                "oscale": np.full((128, 1), 1.0 / s2, dtype=np.float32),
            }
        )
    return in_maps


LAST_RESULT = None


def kernel(**inputs) -> np.ndarray:
    global _program, LAST_RESULT
    _install_wait_split()
    from concourse.bass_utils import run_bass_kernel_spmd

    if _program is None:
        _program = _build_program()
        # Apply the BIR transforms at serialization time so the embedded
        # ant_bir payload (the compile-cache key) reflects them. Both
        # transforms are idempotent, so compile_bir_kernel re-applying
        # them is harmless.
        orig_tjb = _program.to_json_bytes

        def _tjb():
            return _split_multi_waits(_hoist_head_dmas(orig_tjb()))

        _program.to_json_bytes = _tjb

    in_maps = make_in_maps(**inputs)
    res = run_bass_kernel_spmd(
        _program,
        in_maps,
        core_ids=list(range(NCORES)),
        trace=bool(int(os.environ.get("KERNEL_TRACE", "0"))),
    )
    LAST_RESULT = res
    out = np.zeros((T, D), dtype=np.float32)
    for r in res.results:
        out += np.asarray(r["out"]).astype(np.float32)
    return out


# revision 62
# speedup vs baseline: 1.3495x; 1.3495x over previous
"""Trainium2 Bass kernel for nn_CachedMLP (2-expert dense MoE MLP).

Computation (reference):
    ew = expert_weights, swapped if expert_ids[0] != 0
    for e in {0,1}:  down_e = (silu(x @ w1_e.T) * (x @ w3_e.T)) @ w2_e
    out = down_0 * ew[0] + down_1 * ew[1]

Sharding: expert-parallel x tensor-parallel. Core c handles expert c//4
and rows [r*2867, (r+1)*2867) of that expert's w1/w3/w2 (r = c%4),
zero-padded to 2944 = 23*128. ew[e] (and the fp8 weight scale) is
folded into w2 on the host, so the sum of the 8 per-core partial
outputs is the final result.

Precision: w1/w3 are stored as fp8-E3M4 (scaled x64 on host; the PE
upconverts exactly, and silu de-scales via activation's scale=1/64);
w2, x, h stay bf16. Measured end-to-end rel err ~1.6% (tolerance 2e-2).
This halves the w13 HBM traffic: per-core reads drop 71.5 -> ~48 MB,
and the kernel is HBM-wire bound.

Device kernel per core:
  pass 1 over 6 a-slices (512 wide, last 384): for each d-chunk kd,
      gate[t, a-slice] += xT(kd) .T-matmul w1_tile(kd)   (PSUM, N=512)
      up[t, a-slice]   += xT(kd) .T-matmul w3_tile(kd)
    (x is the stationary operand -- loaded once per kd, amortized over
    both matmuls; the moving operand is the fp8 weight tile at N=512,
    so the PE streams near peak instead of reloading weights per MM).
    Then silu(scale=1/64)*up -> h[t, a-slice] (bf16), and PE-transpose
    128-col chunks into hT[a, t] for pass 2.
  pass 2: down[t, d-quarter] += hT(ka).T-matmul w2_tile(ka, q),
    accumulated over all 23 ka in 2 PSUM banks per 1024-wide quarter,
    then copied to SBUF and DMA'd out as bf16 partials.

DMA ring: everything data-sized (x, w13, w2, out) rides the single SP
(sync) HWDGE ring, so program-emission order IS the wire priority
order -- per slice: w13 groups first, then that slice's w2 quads.
Round-robin between two rings would steal w13 bandwidth during pass 1
(all per-core queues share one DMA engine anyway).
"""

import json
import os

import ml_dtypes
import numpy as np

T = 128          # tokens
D = 4096         # hidden dim
ACTIVE = 11468   # sparsity-selected neurons per expert
NCORES = 8
ASH = ACTIVE // 4        # 2867 active rows per core
NKA = 23                 # 128-row a-chunks per core
APAD = NKA * 128         # 2944
NKD = D // 128           # 32 d-chunks
JW_LAST = ASH - (NKA - 1) * 128  # 51 useful rows in the last a-chunk
NSL = 6                  # a-slices in pass 1
SLW = 512                # slice width (last slice: 384)
SLW_LAST = APAD - 5 * SLW  # 384
NQ = 4                   # pass-2 d-quarters
QW = D // NQ             # 1024

WSCALE = 64.0            # fp8 weight scale for w1/w3
K8 = 14                  # leading 128-row a-chunks of w2 stored e3m4
KB16 = NKA - K8          # trailing chunks kept bf16 (8 paired + last jw=51)

BF16 = ml_dtypes.bfloat16
E3M4 = ml_dtypes.float8_e3m4

_EVENTSEM_CAP = 2


def _split_multi_waits(bir_json: bytes) -> bytes:
    """Hoist excess per-instruction sync waits into standalone waits.

    The axon-path walrus build accepts at most 1 sync-wait command per
    instruction (2 for EventSemaphore); Tile's wait assigner can emit
    more. Extra waits become wait-only EventSemaphore instructions
    inserted just before the offender on the same engine stream, which
    preserves semantics (the engine would have blocked there anyway).
    """
    d = json.loads(bir_json)
    for func in d.get("functions", []):
        for blk in func.get("blocks", []):
            out = []
            for inst in blk.get("instructions", []):
                sync = inst.get("sync_info")
                waits = (sync or {}).get("on_wait") or []
                cap = _EVENTSEM_CAP if inst.get("opcode") == "EventSemaphore" else 1
                if len(waits) > cap:
                    extra, keep = waits[:-cap], waits[-cap:]
                    for j in range(0, len(extra), _EVENTSEM_CAP):
                        w_inst = {
                            "engine": inst["engine"],
                            "ins": [],
                            "name": f"{inst['name']}-hw{j}",
                            "opcode": "EventSemaphore",
                            "outs": [],
                            "sync_info": {
                                "on_update": [],
                                "on_wait": extra[j : j + _EVENTSEM_CAP],
                            },
                        }
                        if "debug" in inst:
                            w_inst["debug"] = inst["debug"]
                        out.append(w_inst)
                    sync["on_wait"] = keep
                out.append(inst)
            blk["instructions"] = out
    return json.dumps(d).encode()


def _hoist_head_dmas(bir_json: bytes, max_hoist: int = 4) -> bytes:
    """Move the leading wait-free SP DMACopies to the head of the preamble.

    Tile's prologue (engine reg-init + const memsets + all-engine
    barrier rounds) takes ~8.5us before the first dma_start issues,
    leaving the HBM wire idle. The first input DMAs have no waits and
    their destination tiles are disjoint from everything the preamble
    writes (asserted below), so issuing them at the very front of the
    SP stream is safe: DGE rings and DMA semaphores are initialized by
    the runtime at NEFF load (not by the preamble), HWDGE keeps
    per-engine FIFO order, and the sem increments are only ever waited
    on with >= thresholds.
    """
    d = json.loads(bir_json)
    for func in d.get("functions", []):
        blocks = func.get("blocks", [])
        if len(blocks) < 2:
            continue
        main, tile_blk = blocks[0], blocks[1]
        if main.get("name") != "main" or not tile_blk.get("name", "").startswith(
            "tile_context"
        ):
            continue
        # preamble must write only const tiles, else hoisting is unsafe
        pre_outs = {
            o.get("memref")
            for inst in main["instructions"]
            for o in inst.get("outs", [])
            if isinstance(o, dict)
        }
        if any(m and not m.startswith("const-") for m in pre_outs):
            continue
        hoisted = []
        remaining = []
        for inst in tile_blk["instructions"]:
            if (
                len(hoisted) < max_hoist
                and inst.get("engine") == "SP"
                and inst.get("opcode") == "DMACopy"
                and not ((inst.get("sync_info") or {}).get("on_wait"))
            ):
                hoisted.append(inst)
            else:
                remaining.append(inst)
        if not hoisted:
            continue
        main["instructions"][0:0] = hoisted
        tile_blk["instructions"] = remaining
    return json.dumps(d).encode()


def _install_wait_split():
    import concourse.bass2jax as b2j
    import concourse.bass_utils as bu

    if getattr(bu.compile_bir_kernel, "_wait_split", False):
        return
    orig = bu.compile_bir_kernel

    def compile_with_split(bir_json, tmpdir, neff_name="file.neff"):
        return orig(_split_multi_waits(_hoist_head_dmas(bir_json)), tmpdir, neff_name)

    compile_with_split._wait_split = True
    bu.compile_bir_kernel = compile_with_split
    if getattr(b2j, "compile_bir_kernel", None) is orig:
        b2j.compile_bir_kernel = compile_with_split


_program = None


def _build_program():
    """Build the single-core Bass/Tile program (same program on all 8 cores)."""
    import concourse.bass as bass
    import concourse.mybir as mybir
    from concourse.tile import TileContext

    f32 = mybir.dt.float32
    bf16 = mybir.dt.bfloat16
    fp8e3 = mybir.dt.float8e3
    Silu = mybir.ActivationFunctionType.Silu

    nc = bass.Bass()
    xb = nc.declare_dram_parameter("xb", [128, D], bf16, isOutput=False)
    ident = nc.declare_dram_parameter("ident", [128, 128], bf16, isOutput=False)
    w13a = nc.declare_dram_parameter(
        "w13a", [NSL - 1, NKD // 8, 128, 16 * SLW], fp8e3, isOutput=False
    )
    w13b = nc.declare_dram_parameter(
        "w13b", [NKD // 8, 128, 16 * SLW_LAST], fp8e3, isOutput=False
    )
    w2q8q = nc.declare_dram_parameter(
        "w2q8q", [NQ, 3, 128, 4 * QW], fp8e3, isOutput=False
    )
    w2q8p = nc.declare_dram_parameter(
        "w2q8p", [NQ, 128, 2 * QW], fp8e3, isOutput=False
    )
    w2qbq = nc.declare_dram_parameter(
        "w2qbq", [NQ, 2, 128, 4 * QW], bf16, isOutput=False
    )
    w2ql = nc.declare_dram_parameter("w2ql", [NQ, 128, QW], bf16, isOutput=False)
    oscale = nc.declare_dram_parameter("oscale", [128, 1], f32, isOutput=False)
    out = nc.declare_dram_parameter("out", [T, D], bf16, isOutput=True)

    def slw_of(s):
        return SLW_LAST if s == NSL - 1 else SLW

    def jw_of(ka):
        return JW_LAST if ka == NKA - 1 else 128

    with TileContext(nc) as tc:
        with (
            tc.tile_pool(name="singles", bufs=1) as singles,
            tc.tile_pool(name="w13p", bufs=4) as w13p,
            tc.tile_pool(name="w2p", bufs=14) as w2p,
            tc.tile_pool(name="gap", bufs=2) as gap,
            tc.tile_pool(name="hp", bufs=2) as hp,
            tc.tile_pool(name="outp", bufs=2) as outp,
            tc.tile_pool(name="psum_ug", bufs=1, space="PSUM") as psum_ug,
            tc.tile_pool(name="psum_t", bufs=2, space="PSUM") as psum_t,
            tc.tile_pool(name="psum_o", bufs=2, space="PSUM") as psum_o,
        ):
            # x on the SP ring, split into quarters: d-chunk group g only
            # needs quarter g, so quarter 0 goes first and quarters 1-3
            # are interleaved ahead of their w13 group in slice 0 below.
            # This gets the first matmul going after ~1.3 MB of wire.
            xb_s = singles.tile([128, D], bf16)

            def xb_quarter(xq):
                nc.sync.dma_start(
                    out=xb_s[:, xq * (D // 4) : (xq + 1) * (D // 4)],
                    in_=xb[:, xq * (D // 4) : (xq + 1) * (D // 4)],
                )

            xb_quarter(0)
            osc_s = singles.tile([128, 1], f32)
            nc.scalar.dma_start(out=osc_s, in_=oscale[:, :])
            id_s = singles.tile([128, 128], bf16)
            nc.scalar.dma_start(out=id_s, in_=ident[:, :])
            hT = singles.tile([128, APAD], bf16)

            # pass-2 plumbing, shared between the interleaved quarter 0
            # (emitted inside pass 1 as its hT chunks appear) and the
            # trailing quarters 1-3. w2 arrives in 4-ka quads aligned to
            # pass-1 slices; the two bf16 quads straddle slices, so tiles
            # are held across slice boundaries in `held`.
            w2cache = {}
            held = {}

            def w2dma(q, kind, idx):
                if kind == "e4":
                    t = w2p.tile([128, 4 * QW], fp8e3)
                    nc.sync.dma_start(out=t, in_=w2q8q[q, idx, :, :])
                elif kind == "ep":
                    t = w2p.tile([128, 2 * QW], fp8e3)
                    nc.sync.dma_start(out=t, in_=w2q8p[q, :, :])
                elif kind == "b4":
                    t = w2p.tile([128, 4 * QW], bf16)
                    nc.sync.dma_start(out=t, in_=w2qbq[q, idx, :, :])
                else:  # "last" (ka 22, jw rows only)
                    t = w2p.tile([128, QW], bf16)
                    nc.sync.dma_start(out=t[:JW_LAST], in_=w2ql[q, :JW_LAST, :])
                return t

            SLICE_FETCH = {
                0: [("e4", 0)],
                1: [("e4", 1)],
                2: [("e4", 2)],
                3: [("ep", 0), ("b4", 0)],
                4: [("b4", 1)],
                5: [("last", 0)],
            }

            def get_tile(q, kind, idx):
                return w2cache.pop((q, kind, idx), None) or w2dma(q, kind, idx)

            def prefetch_slice(q, s):
                for kind, idx in SLICE_FETCH[s]:
                    w2cache[(q, kind, idx)] = w2dma(q, kind, idx)

            def emit_slice_mms(q, ops, s):
                def mm2(w2t, ka, colbase):
                    lhsT = hT[:, ka * 128 : (ka + 1) * 128]
                    for b in range(2):
                        nc.tensor.matmul(
                            ops[:, b * 512 : (b + 1) * 512],
                            lhsT,
                            w2t[:, colbase + b * 512 : colbase + (b + 1) * 512],
                            start=(ka == 0),
                            stop=False,
                            skip_group_check=True,
                        )

                if s <= 2:
                    t = get_tile(q, "e4", s)
                    for m in range(4):
                        mm2(t, 4 * s + m, m * QW)
                elif s == 3:
                    t = get_tile(q, "ep", 0)
                    mm2(t, 12, 0)
                    mm2(t, 13, QW)
                    tb = get_tile(q, "b4", 0)
                    held[(q, 0)] = tb
                    mm2(tb, 14, 0)
                    mm2(tb, 15, QW)
                elif s == 4:
                    tb = held.pop((q, 0))
                    mm2(tb, 16, 2 * QW)
                    mm2(tb, 17, 3 * QW)
                    tb1 = get_tile(q, "b4", 1)
                    held[(q, 1)] = tb1
                    mm2(tb1, 18, 0)
                    mm2(tb1, 19, QW)
                else:
                    tb1 = held.pop((q, 1))
                    mm2(tb1, 20, 2 * QW)
                    mm2(tb1, 21, 3 * QW)
                    tl = get_tile(q, "last", 0)
                    lhsT = hT[:JW_LAST, (NKA - 1) * 128 : NKA * 128]
                    for b in range(2):
                        nc.tensor.matmul(
                            ops[:, b * 512 : (b + 1) * 512],
                            lhsT,
                            tl[:JW_LAST, b * 512 : (b + 1) * 512],
                            start=False,
                            stop=True,
                            skip_group_check=True,
                        )

            def emit_store(q, ops):
                oth = outp.tile([T, QW], bf16, name=f"oth{q}", tag="oth")
                nc.vector.tensor_scalar_mul(out=oth, in0=ops, scalar1=osc_s)
                # output store on the SP ring (idle once w13 is done):
                # doesn't queue behind the remaining w2 transfers on ACT
                nc.sync.dma_start(out=out[:, q * QW : (q + 1) * QW], in_=oth)

            # pass 1: gate/up in [t, a] layout, x stationary, fp8 w13 moving.
            # w13 arrives in 8-kd batches (1 MB, 8 KB partition lines).
            # Quarter 0 of the down projection rides along: its ka-pair
            # matmuls are emitted right after the slice that produced
            # their hT chunks, filling pass 1's DMA-wait gaps on the PE.
            ops_q0 = psum_o.tile([128, QW], f32, name="o0", tag="od")
            ops_q1 = psum_o.tile([128, QW], f32, name="o1", tag="od")
            for s in range(NSL):
                sw = slw_of(s)
                gate_ps = psum_ug.tile([128, sw], f32, name=f"g{s}", tag="gate")
                up_ps = psum_ug.tile([128, sw], f32, name=f"u{s}", tag="up")
                for g in range(NKD // 8):
                    if s == 0 and g >= 1:
                        xb_quarter(g)
                    w13t = w13p.tile([128, 16 * sw], fp8e3)
                    if s == 0 and g == 0:
                        # split the very first tile so the first matmuls
                        # start after 512 KB instead of 1 MB
                        hw_ = 8 * sw
                        nc.sync.dma_start(
                            out=w13t[:, :hw_], in_=w13a[0, 0, :, :hw_]
                        )
                        nc.sync.dma_start(
                            out=w13t[:, hw_:], in_=w13a[0, 0, :, hw_:]
                        )
                    elif s < NSL - 1:
                        nc.sync.dma_start(out=w13t, in_=w13a[s, g, :, :])
                    else:
                        nc.sync.dma_start(out=w13t, in_=w13b[g, :, :])
                    for m in range(8):
                        kd = 8 * g + m
                        lhsT = xb_s[:, kd * 128 : (kd + 1) * 128]
                        nc.tensor.matmul(
                            gate_ps,
                            lhsT,
                            w13t[:, m * 2 * sw : m * 2 * sw + sw],
                            start=(kd == 0),
                            stop=(kd == NKD - 1),
                        )
                        nc.tensor.matmul(
                            up_ps,
                            lhsT,
                            w13t[:, m * 2 * sw + sw : (m + 1) * 2 * sw],
                            start=(kd == 0),
                            stop=(kd == NKD - 1),
                        )
                ga = gap.tile([128, sw], f32)
                nc.scalar.activation(
                    out=ga, in_=gate_ps, func=Silu, scale=1.0 / WSCALE
                )
                hsl = hp.tile([128, sw], bf16)
                nc.vector.tensor_mul(out=hsl, in0=ga, in1=up_ps)
                # hT chunks via PE transpose + DVE copy. Not the DMA XBAR
                # (1.25us of DMA-engine time per chunk on the kernel's
                # scarcest resource) and not DVE 32x32 block transposes
                # (16 small DVE ops per chunk; per-instruction overhead
                # measured ~45us slower end-to-end).
                for c in range(sw // 128):
                    ka = s * 4 + c
                    tp = psum_t.tile([128, 128], bf16, name=f"t{ka}", tag="tp")
                    nc.tensor.transpose(tp, hsl[:, c * 128 : (c + 1) * 128], id_s)
                    nc.vector.tensor_copy(
                        out=hT[:, ka * 128 : (ka + 1) * 128], in_=tp
                    )
                # quarters 0 and 1 of the down projection ride inside
                # pass 1 (PE work to fill DMA-wait gaps, and their w2
                # consumption keeps the wire schedule balanced)
                emit_slice_mms(0, ops_q0, s)
                emit_slice_mms(1, ops_q1, s)
            emit_store(0, ops_q0)
            emit_store(1, ops_q1)

            # pass 2 remainder: quarters 2-3, each accumulating across all
            # ka in 2 PSUM banks (rotating through the bufs=2 pool), output
            # de-scale copy + DMA overlapping the next quarter's compute.
            for q in range(2, NQ):
                ops = psum_o.tile([128, QW], f32, name=f"o{q}", tag="od")
                for s in range(NSL):
                    emit_slice_mms(q, ops, s)
                emit_store(q, ops)

    return nc


def _pack_w13(w1s: np.ndarray, w3s: np.ndarray):
    """[ASH, D] f32 pair -> ([NSL-1, 4, 128, 16*SLW], [4, 128, 16*SLW_LAST])
    e3m4 blobs, scaled by WSCALE, batched 8 d-chunks per 1 MiB transfer
    (8 KB partition lines, the shape that sustains full DMA throughput).

    blob[s, g, p, m*2*sw + w*sw + j] = w{1,3}s[s*SLW + j, (8g+m)*128 + p].
    """
    padded = np.zeros((2, NSL * SLW, D), dtype=E3M4)
    padded[0, :ASH] = (w1s * np.float32(WSCALE)).astype(E3M4)
    padded[1, :ASH] = (w3s * np.float32(WSCALE)).astype(E3M4)
    # [2, NSL, SLW, NKD, 128] -> [s, kd, p, w, j]
    r = padded.reshape(2, NSL, SLW, NKD, 128).transpose(1, 3, 4, 0, 2)
    # [s, g, m, p, w, j] -> [s, g, p, m, w, j]
    r = r.reshape(NSL, NKD // 8, 8, 128, 2, SLW).transpose(0, 1, 3, 2, 4, 5)
    a = np.ascontiguousarray(r[: NSL - 1]).reshape(NSL - 1, NKD // 8, 128, 16 * SLW)
    b = np.ascontiguousarray(r[NSL - 1, :, :, :, :, :SLW_LAST]).reshape(
        NKD // 8, 128, 16 * SLW_LAST
    )
    return a, b


def _pack_w2(w2s: np.ndarray, scale: float):
    """[ASH, D] f32 -> (e3m4 4-ka quads, e3m4 2-ka pair, bf16 4-ka quads,
    last bf16 chunk) blobs, layout [q, idx, p, m*QW + j].

    scale (= ew * S2 / WSCALE) is folded in; ka < 12 e3m4 quads, ka 12-13
    an e3m4 pair, ka 14-21 bf16 quads, the last (jw=51) chunk on its own
    so its DMA can be row-trimmed.
    """
    v = w2s * np.float32(scale)
    eq = v[: 12 * 128].astype(E3M4)
    a = np.ascontiguousarray(
        eq.reshape(3, 4, 128, NQ, QW).transpose(3, 0, 2, 1, 4)
    ).reshape(NQ, 3, 128, 4 * QW)
    ep = v[12 * 128 : 14 * 128].astype(E3M4)
    p = np.ascontiguousarray(
        ep.reshape(2, 128, NQ, QW).transpose(2, 1, 0, 3)
    ).reshape(NQ, 128, 2 * QW)
    bq = v[14 * 128 : 22 * 128].astype(BF16)
    b = np.ascontiguousarray(
        bq.reshape(2, 4, 128, NQ, QW).transpose(3, 0, 2, 1, 4)
    ).reshape(NQ, 2, 128, 4 * QW)
    last = np.zeros((128, D), dtype=BF16)
    last[:JW_LAST] = v[(NKA - 1) * 128 :].astype(BF16)
    l = np.ascontiguousarray(last.reshape(128, NQ, QW).transpose(1, 0, 2))
    return a, p, b, l


def _pack_x(x: np.ndarray) -> np.ndarray:
    """[T, D] f32 -> [128, D] bf16: xb[p, kd*128 + t] = x[t, kd*128 + p]."""
    return (
        x.astype(BF16).reshape(T, NKD, 128).transpose(2, 1, 0).reshape(128, NKD * T)
    )


def make_in_maps(
    hidden_states,
    expert_weights,
    expert_ids,
    w1_e0,
    w3_e0,
    w2_e0,
    w1_e1,
    w3_e1,
    w2_e1,
):
    ids = np.asarray(expert_ids).reshape(-1)
    ew = np.asarray(expert_weights, dtype=np.float32).reshape(-1)
    if int(ids[0]) != 0:
        ew = ew[::-1]

    xb = _pack_x(np.asarray(hidden_states, dtype=np.float32))
    ident = np.eye(128, dtype=BF16)
    w1 = (np.asarray(w1_e0, np.float32), np.asarray(w1_e1, np.float32))
    w3 = (np.asarray(w3_e0, np.float32), np.asarray(w3_e1, np.float32))
    w2 = (np.asarray(w2_e0, np.float32), np.asarray(w2_e1, np.float32))

    in_maps = []
    for core in range(NCORES):
        e, r = divmod(core, 4)
        rows = slice(r * ASH, (r + 1) * ASH)
        w13a, w13b = _pack_w13(w1[e][rows], w3[e][rows])
        # h is scaled by WSCALE (up de-scale folded here), so w2 gets
        # ew / WSCALE, plus S2 to center the e3m4 chunks in range
        # (sigma_eff ~ 2); S2 is undone by oscale on the output copy.
        ewe = float(ew[e])
        s2 = 2.0 ** np.round(np.log2(6400.0 / max(abs(ewe), 1e-6)))
        w2q8q, w2q8p, w2qbq, w2ql = _pack_w2(w2[e][rows], ewe * s2 / WSCALE)
        in_maps.append(
            {
                "xb": xb,
                "ident": ident,
                "w13a": w13a,
                "w13b": w13b,
                "w2q8q": w2q8q,
                "w2q8p": w2q8p,
                "w2qbq": w2qbq,
                "w2ql": w2ql,
                "oscale": np.full((128, 1), 1.0 / s2, dtype=np.float32),
            }
        )
    return in_maps


LAST_RESULT = None


def kernel(**inputs) -> np.ndarray:
    global _program, LAST_RESULT
    _install_wait_split()
    from concourse.bass_utils import run_bass_kernel_spmd

    if _program is None:
        _program = _build_program()
        # Apply the BIR transforms at serialization time so the embedded
        # ant_bir payload (the compile-cache key) reflects them. Both
        # transforms are idempotent, so compile_bir_kernel re-applying
        # them is harmless.
        orig_tjb = _program.to_json_bytes

        def _tjb():
            return _split_multi_waits(_hoist_head_dmas(orig_tjb()))

        _program.to_json_bytes = _tjb

    in_maps = make_in_maps(**inputs)
    res = run_bass_kernel_spmd(
        _program,
        in_maps,
        core_ids=list(range(NCORES)),
        trace=bool(int(os.environ.get("KERNEL_TRACE", "0"))),
    )
    LAST_RESULT = res
    out = np.zeros((T, D), dtype=np.float32)
    for r in res.results:
        out += np.asarray(r["out"]).astype(np.float32)
    return out
